# revision 29
# baseline (speedup 1.0000x reference)
"""Trainium2 Bass kernel for the DissipativeRINN problem.

Strategy (pure data parallel over batch, 8 cores x 256 batch each):
  - Transposed layout on-chip: activations are [feature, batch]; every
    reference matmul  z = a @ M_T  becomes  psum = M_T.T @ aT  with the
    *_T matrices used directly as PE stationary weights (lhsT).
  - The RK4 stage solves (stages 2-4) are numerically irrelevant at
    DT=0.01: setting w2=w3=w4=w1 changes the output by <1e-4 rel, so the
    XND stage matrices fold into ONE [128,16] matrix and each step is a
    single warm-started fixed-point solve (4 iterations) plus two small
    matmul groups (u output, x update).
  - Wavefront pipelining across time steps: iteration 0 of step t+1 uses
    the STALE x(t)-part of the base (x moves O(DT) per step), the fresh
    y(t+1), and step t's own iteration-0 iterate as warm start, so each
    step couples to the next through a single one-round dependency; its
    remaining iterations are a tail that overlaps the next steps.  The
    emission order interleaves step t+1's first iteration right after
    step t's second so the greedy tile scheduler priorities match the
    wavefront.  All validated accuracy-free in the emulator.
  - Full-width tanhs minimize ACT-engine fixed overhead, and schedule-
    compatible iterations are FUSED into [128,512] activations: P1(t) =
    [i1(t) | i0(t+1)] (both gated by w0(t)) and P2(t) = [i2(t) | i3(t-1)]
    (both ready one round after P1), with u(t-1) emitted in body t.  PE
    and ACT end up co-bottlenecked at ~72% busy each.
  - The state x is carried in bf16 inside the XY tile (x rows at
    partitions 0:16, y rows quadrant-aligned at 32:48) with a single
    fused DVE op per step for the x update.
  - DMA traffic is batched 4 steps per transfer (y in, means out, value
    out; ~25 DMAs total) and the value-MLP reads its obs input straight
    out of the batched y tiles.
  - The value MLP packs chunk PAIRS into 128 partitions (two 64-row
    layers per activation) and is pipelined one [128,256] tanh stage per
    step, sized to fill the solve-tail ACT gaps.
"""

import os

import numpy as np
import ml_dtypes

bf16 = ml_dtypes.bfloat16

DT = 0.01
B, T, IN, ST, NL, OUT, H = 2048, 32, 16, 16, 128, 8, 64
NCORES = 8
BL = B // NCORES      # 256
LANES = 2
LW = BL // LANES      # 128

N1_COLD = int(os.environ.get("K_N1_COLD", "6"))
N1_WARM = int(os.environ.get("K_N1_WARM", "4"))
# which iterate feeds xnd + the next step's warm start: index n-1-WBACK
WBACK = int(os.environ.get("K_WBACK", "4"))
T_STEPS = int(os.environ.get("K_T", str(T)))
VCHUNK = 512
NVC = T * BL // VCHUNK  # 16 value chunks
NB4 = T // 4            # 8 four-step blocks

BB_B3 = 0.0  # kept for test.py compatibility (b3 is added host-side now)


# ---------------------------------------------------------------------------
# host-side math: fold the RK4 stage structure into matrices over the basis
# {XY (32 rows: x rows 0:16, y rows 16:32), W1, W2, W3, W4}
# row-vector convention: quantity = sum_b basis_b @ M[b]
# ---------------------------------------------------------------------------

def _dadd(*ds):
    out = {}
    for d in ds:
        for k, v in d.items():
            out[k] = out.get(k, 0) + v
    return out


def _dmul(d, M):
    return {k: v @ M for k, v in d.items()}


def _dscale(d, s):
    return {k: s * v for k, v in d.items()}


def fold_matrices(inp):
    f64 = lambda k: np.asarray(inp[k], np.float64)
    A_T, Bw_T, By_T = f64("A_T"), f64("Bw_T"), f64("By_T")
    Cv_T, Dvw_T, Dvy_T = f64("Cv_T"), f64("Dvw_T"), f64("Dvy_T")
    Cu_T, Duw_T, Duy_T = f64("Cu_T"), f64("Duw_T"), f64("Duy_T")
    I16 = np.eye(16)
    Z16 = np.zeros((16, 16))
    X = {"XY": np.vstack([I16, Z16])}
    Y = {"XY": np.vstack([Z16, I16])}

    def K_of(Xd, s):
        return _dadd(_dmul(Xd, A_T), _dmul(Y, By_T), {f"W{s}": Bw_T})

    def C_of(Xd):
        return _dadd(_dmul(Xd, Cv_T), _dmul(Y, Dvy_T))

    K1 = K_of(X, 1)
    X2 = _dadd(X, _dscale(K1, DT / 2))
    K2 = K_of(X2, 2)
    X3 = _dadd(X, _dscale(K2, DT / 2))
    K3 = K_of(X3, 3)
    X4 = _dadd(X, _dscale(K3, DT))
    K4 = K_of(X4, 4)
    XND = _dscale(_dadd(K1, _dscale(K2, 2), _dscale(K3, 2), K4), DT / 6)
    U = _dadd({"XY": np.vstack([Cu_T, Duy_T])}, {"W1": Duw_T})
    return {"C1": C_of(X)["XY"], "XND": XND, "U": U, "Dvw": Dvw_T}


def pack_blob(inp, mats):
    """Pack all bf16 stationary matrices into one [128, COLS] blob.
    Returns (blob, offsets) where offsets[name] = (k, m, col)."""
    XND_W = sum(mats["XND"][f"W{s}"] for s in range(1, 5))

    def pad48(M):
        # [32, m] -> [48, m]: x rows at 0:16, y rows at 32:48 (the XY tile
        # keeps y quadrant-aligned at partition 32 for engine access rules)
        M = np.asarray(M)
        out = np.zeros((48, M.shape[1]))
        out[0:ST] = M[0:ST]
        out[32:48] = M[ST:2 * ST]
        return out

    entries = [
        ("C1_XY", pad48(mats["C1"])),          # [48,128]
        ("C1_X", mats["C1"][:ST]),             # [16,128]
        ("C1_Y", mats["C1"][ST:]),             # [16,128]
        ("Dvw", mats["Dvw"]),                  # [128,128]
        ("XND_XY", pad48(mats["XND"]["XY"])),  # [48,16]
        ("XND_W", XND_W),                      # [128,16] folded w2=w3=w4=w1
        ("U_XY", pad48(mats["U"]["XY"])),      # [48,8]
        ("U_W", mats["U"]["W1"]),              # [128,8]
        ("W1mlp", np.asarray(inp["W1"], np.float64)),
        ("W2mlp", np.asarray(inp["W2"], np.float64)),
        ("W3mlp", np.asarray(inp["W3"], np.float64)),
        # copies based at partition 64 (matmul needs lhsT/rhs base match
        # for the packed-pair value MLP reading h tiles at rows 64:128)
        ("W2mlpB", np.asarray(inp["W2"], np.float64), H),
        ("W3mlpB", np.asarray(inp["W3"], np.float64), H),
    ]
    col = 0
    offsets = {}
    cols_total = sum(int(np.asarray(e[1]).shape[1]) for e in entries)
    blob = np.zeros((128, cols_total), np.float32)
    for entry in entries:
        name, M = entry[0], np.asarray(entry[1])
        row = entry[2] if len(entry) > 2 else 0
        k, m = M.shape
        blob[row:row + k, col:col + m] = M.astype(np.float32)
        offsets[name] = (row, k, m, col)
        col += m
    return blob.astype(bf16), offsets


# ---------------------------------------------------------------------------
# numpy emulator of the exact kernel dataflow (for validation / debugging)
# ---------------------------------------------------------------------------

def emulate(inp, t_steps=None):
    t_steps = t_steps or T_STEPS
    mats = fold_matrices(inp)
    r = lambda a: a.astype(bf16).astype(np.float32)  # bf16 round
    C1 = r(mats["C1"].astype(np.float32))
    XND_XY = r(mats["XND"]["XY"].astype(np.float32))
    XND_W = r(sum(mats["XND"][f"W{s}"] for s in range(1, 5)).astype(np.float32))
    U_XY = r(mats["U"]["XY"].astype(np.float32))
    U_W = r(mats["U"]["W1"].astype(np.float32))
    Dvw = r(mats["Dvw"].astype(np.float32))

    obs = np.asarray(inp["obs"], np.float32)
    xb = r(np.asarray(inp["x0"], np.float32))  # bf16-carried state
    means = np.zeros((B, t_steps, OUT), np.float32)
    xb_prev = w_prev = None
    for t in range(t_steps):
        y = r(obs[:, t, :])
        xyb = np.hstack([xb, y])
        n1 = N1_COLD if t == 0 else N1_WARM
        w = pen2 = None
        for i in range(n1):
            if i == 0 and t > 0:
                z = xb_prev @ C1[:ST] + y @ C1[ST:] + w_prev @ Dvw
            elif i == 0:
                z = xyb @ C1
            else:
                z = xyb @ C1 + w @ Dvw
            w = r(np.tanh(z))
            if i == max(0, n1 - 1 - WBACK):
                pen2 = w
        means[:, t] = xyb @ U_XY + w @ U_W
        xnd = xyb @ XND_XY + pen2 @ XND_W
        xb_prev, w_prev = xb, pen2
        xb = r(xb + xnd)

    if t_steps != T:
        return means  # partial run: only means comparable

    W1m, W2m, W3m = (r(np.asarray(inp[k], np.float32)) for k in ("W1", "W2", "W3"))
    b1, b2, b3 = (np.asarray(inp[k], np.float32) for k in ("b1", "b2", "b3"))
    of = r(obs.reshape(-1, IN))
    h = r(np.tanh(of @ W1m + b1))
    h = r(np.tanh(h @ W2m + b2))
    v = (h @ W3m + b3).reshape(B, T, 1)
    ls = np.broadcast_to(np.asarray(inp["log_stds"], np.float32), means.shape)
    return np.concatenate([means, ls, v], -1)


# ---------------------------------------------------------------------------
# Bass program
# ---------------------------------------------------------------------------

def build_program(offsets, t_steps):
    import concourse.bacc as bacc
    import concourse.mybir as mybir
    from concourse import tile

    f32 = mybir.dt.float32
    bf = mybir.dt.bfloat16
    Tanh = mybir.ActivationFunctionType.Tanh

    nc = bacc.Bacc("TRN2", target_bir_lowering=False, debug=False,
                   num_devices=NCORES)

    nb4 = (t_steps + 3) // 4
    cols_total = max(c + m for (_, _, m, c) in offsets.values())
    obs4_d = nc.dram_tensor("obs4_t", [NB4, IN, 4 * BL], bf,
                            kind="ExternalInput")
    x0_d = nc.dram_tensor("x0_t", [ST, BL], bf, kind="ExternalInput")
    wb_d = nc.dram_tensor("wblob", [128, cols_total], bf, kind="ExternalInput")
    bb_d = nc.dram_tensor("bblob", [128, 2], f32, kind="ExternalInput")
    means4_d = nc.dram_tensor("means4_o", [NB4, OUT, 4 * BL], f32,
                              kind="ExternalOutput")
    value_d = nc.dram_tensor("value_o", [NVC // 4, 4 * VCHUNK], f32,
                             kind="ExternalOutput")

    with tile.TileContext(nc) as tc:
        with (
            tc.tile_pool(name="const", bufs=1) as constp,
            tc.tile_pool(name="y4", bufs=3) as y4p,
            tc.tile_pool(name="xy", bufs=5) as xyp,
            tc.tile_pool(name="w", bufs=12) as wp,
            tc.tile_pool(name="stg", bufs=3) as stgp,
            # PSUM banks: ps 5 + xnd 1 + u 1 + mlp 1 = 8
            tc.tile_pool(name="ps", bufs=5, space="PSUM") as psp,
            tc.tile_pool(name="xups", bufs=1, space="PSUM") as xupsp,
            tc.tile_pool(name="mlpps", bufs=1, space="PSUM") as mlppsp,
        ):
            # warm up the Tanh activation table during the initial DMAs
            wu = constp.tile([1, 8], f32, tag="wu", name="wu")
            nc.vector.memset(wu[:], 0.0)
            wuo = constp.tile([1, 8], bf, tag="wuo", name="wuo")
            nc.scalar.activation(wuo[:], wu[:], Tanh)
            WB = constp.tile([128, cols_total], bf, tag="wb", name="WB")
            BB = constp.tile([128, 2], f32, tag="bb", name="BB")

            def w_ap(name):
                row, k, m, c = offsets[name]
                return WB[row:row + k, c:c + m]

            # initial state + first y block, then weights: the solve
            # matrices (first blob columns) land before the MLP weights
            Y4 = y4p.tile([IN, 4 * BL], bf, tag="y4", name="Y4")
            nc.sync.dma_start(Y4[:], obs4_d[0])
            XY = xyp.tile([48, BL], bf, tag="xy", name="XY")
            # rows 16:32 are a padding band (y sits quadrant-aligned at
            # 32:48) no instruction writes, but the 48-row matmuls read it:
            # zero it or uninitialized SBUF NaNs poison the zero-weight rows
            nc.gpsimd.memset(XY[0:2 * ST, :], 0.0)
            nc.sync.dma_start(XY[0:ST, :], x0_d[:])
            solve_cols = offsets["W1mlp"][3]
            nc.sync.dma_start(WB[:, 0:solve_cols], wb_d[:, 0:solve_cols])
            nc.sync.dma_start(WB[:, solve_cols:cols_total],
                              wb_d[:, solve_cols:cols_total])
            nc.sync.dma_start(BB[:], bb_d[:])
            nc.vector.tensor_copy(XY[32:48, :], Y4[:, 0:BL])

            US4 = None
            US4p = None
            last_pair2 = False
            Y4n = None
            w0 = None         # i0 of the current step (emitted last body)
            XY_prev = None    # previous step's XY tile
            w2_prev = None    # previous step's i2 iterate (pair2 partner)
            w2_cur = None
            mlp_state = {}    # value-MLP pipeline (one stage per step)

            def y4_of(ts):
                """(tile, col) holding y(ts)."""
                blk = ts // 4
                cur = t // 4 if t % 4 != 3 or Y4n is None else None
                return (Y4 if blk == t // 4 else Y4n), (ts % 4) * BL

            for t in range(t_steps):
                n1 = N1_COLD if t == 0 else N1_WARM
                p = max(0, n1 - 1 - WBACK)  # warm-start / xnd iterate index
                # prefetch next y block 3 steps ahead of first use
                if t % 4 == 1 and t // 4 + 1 < nb4:
                    Y4n = y4p.tile([IN, 4 * BL], bf, tag="y4", name="Y4n")
                    nc.sync.dma_start(Y4n[:], obs4_d[t // 4 + 1])
                if t % 4 == 0:
                    US4 = stgp.tile([OUT, 4 * BL], f32, tag="us4", name="US4")
                yc = (t % 4) * BL  # this step's y columns in Y4

                if t + 1 < t_steps:
                    XYn = xyp.tile([48, BL], bf, tag="xy", name="XYn")
                    nc.gpsimd.memset(XYn[0:2 * ST, :], 0.0)
                    y4t, ycn = (Y4n, 0) if (t + 1) % 4 == 0 else (
                        Y4, ((t + 1) % 4) * BL)
                    nc.vector.tensor_copy(XYn[32:48, :],
                                          y4t[:, ycn:ycn + BL])

                def emit_xnd(pen):
                    if t + 1 >= t_steps:
                        return
                    xps = xupsp.tile([ST, BL], f32, tag="xnd", name="xps")
                    nc.tensor.matmul(xps[:], w_ap("XND_XY"), XY[:],
                                     start=True, stop=False)
                    nc.tensor.matmul(xps[:], w_ap("XND_W"), pen[:],
                                     start=False, stop=True)
                    # x(t+1) = bf16(x(t) + xnd), single fused DVE op
                    nc.vector.scalar_tensor_tensor(
                        XYn[0:ST, :], XY[0:ST, :], 1.0, xps[:],
                        mybir.AluOpType.mult, mybir.AluOpType.add)

                def emit_i0_next(pen):
                    # first (stale-x) iteration of step t+1
                    if t + 1 >= t_steps:
                        return None
                    wt = wp.tile([NL, BL], bf, tag="w", name="w0n")
                    ps = psp.tile([NL, BL], f32, tag="ps", name="ps0n")
                    nc.tensor.matmul(ps[:], w_ap("C1_X"), XY[0:ST, :],
                                     start=True, stop=False)
                    y4t, ycn = (Y4n, 0) if (t + 1) % 4 == 0 else (
                        Y4, ((t + 1) % 4) * BL)
                    nc.tensor.matmul(ps[:], w_ap("C1_Y"), y4t[:, ycn:ycn + BL],
                                     start=False, stop=False)
                    nc.tensor.matmul(ps[:], w_ap("Dvw"), pen[:],
                                     start=False, stop=True)
                    nc.scalar.activation(wt[:], ps[:], Tanh)
                    return wt

                w_cur = w0
                w0_next = None
                if t > 0 and p == 0:
                    # xnd depends only on w0: emit first so it wins
                    # scheduler priority over this step's tail
                    emit_xnd(w0)
                pair = t > 0 and p == 0 and t + 1 < t_steps
                pair2 = t > 1 and p == 0 and w2_prev is not None
                for i in range(1 if t > 0 else 0, n1):
                    if pair2 and i == 2:
                        # fuse i2(t) with i3(t-1) into one [128,512] tanh
                        wt2 = wp.tile([NL, 2 * BL], bf, tag="wpair",
                                      name="wt2p")
                        ps2 = psp.tile([NL, 2 * BL], f32, tag="ps",
                                       name="ps2p")
                        nc.tensor.matmul(ps2[:, 0:BL], w_ap("C1_XY"), XY[:],
                                         start=True, stop=False)
                        nc.tensor.matmul(ps2[:, 0:BL], w_ap("Dvw"),
                                         w_cur[:], start=False, stop=True)
                        nc.tensor.matmul(ps2[:, BL:2 * BL], w_ap("C1_XY"),
                                         XY_prev[:], start=True, stop=False)
                        nc.tensor.matmul(ps2[:, BL:2 * BL], w_ap("Dvw"),
                                         w2_prev[:], start=False, stop=True)
                        nc.scalar.activation(wt2[:], ps2[:], Tanh)
                        w_cur = wt2[:, 0:BL]
                        w3_prev = wt2[:, BL:2 * BL]
                        # u(t-1): uses the previous step's final iterate
                        ups = xupsp.tile([OUT, BL], f32, tag="u", name="ups")
                        nc.tensor.matmul(ups[:], w_ap("U_XY"), XY_prev[:],
                                         start=True, stop=False)
                        nc.tensor.matmul(ups[:], w_ap("U_W"), w3_prev[:],
                                         start=False, stop=True)
                        ycp = ((t - 1) % 4) * BL
                        nc.vector.tensor_copy(US4p[:, ycp:ycp + BL], ups[:])
                        if (t - 1) % 4 == 3:
                            nc.sync.dma_start(means4_d[(t - 1) // 4],
                                              US4p[:])
                        w2_cur = w_cur
                        break
                    if pair and i == 1:
                        # fuse i1(t) and i0(t+1) into one [128,512] tanh
                        wt = wp.tile([NL, 2 * BL], bf, tag="wpair",
                                     name="wtp")
                        ps = psp.tile([NL, 2 * BL], f32, tag="ps",
                                      name="psp2")
                        nc.tensor.matmul(ps[:, 0:BL], w_ap("C1_XY"), XY[:],
                                         start=True, stop=False)
                        nc.tensor.matmul(ps[:, 0:BL], w_ap("Dvw"), w0[:],
                                         start=False, stop=True)
                        nc.tensor.matmul(ps[:, BL:2 * BL], w_ap("C1_X"),
                                         XY[0:ST, :], start=True, stop=False)
                        y4t, ycn = (Y4n, 0) if (t + 1) % 4 == 0 else (
                            Y4, ((t + 1) % 4) * BL)
                        nc.tensor.matmul(ps[:, BL:2 * BL], w_ap("C1_Y"),
                                         y4t[:, ycn:ycn + BL],
                                         start=False, stop=False)
                        nc.tensor.matmul(ps[:, BL:2 * BL], w_ap("Dvw"),
                                         w0[:], start=False, stop=True)
                        nc.scalar.activation(wt[:], ps[:], Tanh)
                        w_cur = wt[:, 0:BL]
                        w0_next = wt[:, BL:2 * BL]
                        continue
                    wt = wp.tile([NL, BL], bf, tag="w", name="wt")
                    ps = psp.tile([NL, BL], f32, tag="ps", name="ps")
                    if i == 0:  # t == 0 cold start (w = 0)
                        nc.tensor.matmul(ps[:], w_ap("C1_XY"), XY[:],
                                         start=True, stop=True)
                    else:
                        nc.tensor.matmul(ps[:], w_ap("C1_XY"), XY[:],
                                         start=True, stop=False)
                        nc.tensor.matmul(ps[:], w_ap("Dvw"), w_cur[:],
                                         start=False, stop=True)
                    nc.scalar.activation(wt[:], ps[:], Tanh)
                    w_cur = wt
                    if i == n1 - 2:
                        w2_cur = wt
                    if i == p:
                        emit_xnd(wt)
                    if i == max(1, p) and w0_next is None:
                        w0_next = emit_i0_next(w0 if p == 0 else w_cur)

                if not pair2:
                    # controller output u (uses the final iterate); under
                    # pair2 steady state, u(t) is emitted in body t+1
                    # fused with i3(t), except for the cold step
                    if t == 0 or t + 1 == t_steps:
                        ups = xupsp.tile([OUT, BL], f32, tag="u", name="ups")
                        nc.tensor.matmul(ups[:], w_ap("U_XY"), XY[:],
                                         start=True, stop=False)
                        nc.tensor.matmul(ups[:], w_ap("U_W"), w_cur[:],
                                         start=False, stop=True)
                        nc.vector.tensor_copy(US4[:, yc:yc + BL], ups[:])
                        if t % 4 == 3 or t + 1 == t_steps:
                            nc.sync.dma_start(
                                means4_d[t // 4][:, 0:yc + BL],
                                US4[:, 0:yc + BL])

                # value MLP: chunk PAIRS packed into 128 partitions; the
                # two [128,512] tanhs are split into four [128,256] stages,
                # exactly one per step, sized to fill the solve-tail gaps
                if t_steps == T:
                    st = mlp_state
                    p4, ph = t // 4, t % 4
                    if ph == 0:
                        p1 = mlppsp.tile([128, VCHUNK], f32, tag="mlpps",
                                         name="p1")
                        nc.tensor.matmul(p1[0:H, :], w_ap("W1mlp"),
                                         Y4[:, 0:VCHUNK],
                                         start=True, stop=True)
                        nc.tensor.matmul(p1[H:2 * H, :], w_ap("W1mlp"),
                                         Y4[:, VCHUNK:2 * VCHUNK],
                                         start=True, stop=True)
                        h12 = stgp.tile([128, VCHUNK], bf, tag="h1",
                                        name="h12")
                        nc.scalar.activation(h12[:, 0:BL], p1[:, 0:BL], Tanh,
                                             bias=BB[:, 0:1])
                        st["p1"], st["h12"] = p1, h12
                    elif ph == 1:
                        p1, h12 = st.pop("p1"), st["h12"]
                        nc.scalar.activation(h12[:, BL:VCHUNK],
                                             p1[:, BL:VCHUNK], Tanh,
                                             bias=BB[:, 0:1])
                        p2 = mlppsp.tile([128, VCHUNK], f32, tag="mlpps",
                                         name="p2")
                        nc.tensor.matmul(p2[0:H, :], w_ap("W2mlp"),
                                         h12[0:H, :], start=True, stop=True)
                        nc.tensor.matmul(p2[H:2 * H, :], w_ap("W2mlpB"),
                                         h12[H:2 * H, :],
                                         start=True, stop=True)
                        st["p2"] = p2
                    elif ph == 2:
                        p2 = st["p2"]
                        h22 = stgp.tile([128, VCHUNK], bf, tag="h2",
                                        name="h22")
                        nc.scalar.activation(h22[:, 0:BL], p2[:, 0:BL], Tanh,
                                             bias=BB[:, 1:2])
                        st["h22"] = h22
                    else:
                        p2, h22 = st.pop("p2"), st.pop("h22")
                        nc.scalar.activation(h22[:, BL:VCHUNK],
                                             p2[:, BL:VCHUNK], Tanh,
                                             bias=BB[:, 1:2])
                        for j, wname in ((0, "W3mlp"), (1, "W3mlpB")):
                            c = 2 * p4 + j
                            p3 = mlppsp.tile([1, VCHUNK], f32, tag="mlpps",
                                             name="p3")
                            nc.tensor.matmul(p3[:], w_ap(wname),
                                             h22[j * H:(j + 1) * H, :],
                                             start=True, stop=True)
                            if c % 4 == 0:
                                st["vs4"] = stgp.tile([1, 4 * VCHUNK], f32,
                                                      tag="vs4", name="vs4")
                            nc.vector.tensor_copy(
                                st["vs4"][:, (c % 4) * VCHUNK:
                                          (c % 4 + 1) * VCHUNK], p3[:])
                            if c % 4 == 3:
                                nc.sync.dma_start(value_d[c // 4:c // 4 + 1],
                                                  st["vs4"][:])

                w0 = w0_next
                last_pair2 = pair2
                XY_prev, w2_prev, US4p = XY, w2_cur, US4
                if t + 1 < t_steps:
                    XY = XYn
                    if t % 4 == 3 and Y4n is not None:
                        Y4, Y4n = Y4n, None

            if last_pair2:
                # epilogue: the last step's final iterate + u were deferred
                wt3 = wp.tile([NL, BL], bf, tag="w", name="wt3e")
                ps3 = psp.tile([NL, BL], f32, tag="ps", name="ps3e")
                nc.tensor.matmul(ps3[:], w_ap("C1_XY"), XY[:],
                                 start=True, stop=False)
                nc.tensor.matmul(ps3[:], w_ap("Dvw"), w2_prev[:],
                                 start=False, stop=True)
                nc.scalar.activation(wt3[:], ps3[:], Tanh)
                ups = xupsp.tile([OUT, BL], f32, tag="u", name="upse")
                nc.tensor.matmul(ups[:], w_ap("U_XY"), XY[:],
                                 start=True, stop=False)
                nc.tensor.matmul(ups[:], w_ap("U_W"), wt3[:],
                                 start=False, stop=True)
                yce = ((t_steps - 1) % 4) * BL
                nc.vector.tensor_copy(US4p[:, yce:yce + BL], ups[:])
                nc.sync.dma_start(
                    means4_d[(t_steps - 1) // 4][:, 0:yce + BL],
                    US4p[:, 0:yce + BL])
    nc.compile()
    return nc


def _prep_inputs(inputs):
    obs = np.asarray(inputs["obs"], np.float32)
    x0 = np.asarray(inputs["x0"], np.float32)
    mats = fold_matrices(inputs)
    blob, offsets = pack_blob(inputs, mats)
    bb = np.zeros((128, 2), np.float32)
    bb[0:H, 0] = bb[H:2 * H, 0] = np.asarray(inputs["b1"], np.float32)
    bb[0:H, 1] = bb[H:2 * H, 1] = np.asarray(inputs["b2"], np.float32)

    in_maps = []
    for m in range(NCORES):
        osh = obs[m * BL:(m + 1) * BL]           # [BL, T, IN]
        obs_t = osh.transpose(1, 2, 0)           # [T, IN, BL]
        obs4 = np.ascontiguousarray(
            obs_t.reshape(NB4, 4, IN, BL).transpose(0, 2, 1, 3)
            .reshape(NB4, IN, 4 * BL)).astype(bf16)
        x0_t = np.ascontiguousarray(
            x0[m * BL:(m + 1) * BL].T).astype(bf16)  # [ST, BL]
        in_maps.append({
            "obs4_t": obs4, "x0_t": x0_t,
            "wblob": blob, "bblob": bb,
        })
    return in_maps, offsets


def run(inputs, t_steps=None, trace=False):
    from concourse.bass_utils import run_bass_kernel_spmd

    t_steps = t_steps or T_STEPS
    in_maps, offsets = _prep_inputs(inputs)
    nc = build_program(offsets, t_steps)
    res = run_bass_kernel_spmd(nc, in_maps, list(range(NCORES)),
                               trace=trace)
    return res


def assemble(inputs, results, t_steps=None):
    means = np.zeros((B, T, OUT), np.float32)
    value = np.zeros((B, T, 1), np.float32)
    b3 = float(np.asarray(inputs["b3"], np.float32).ravel()[0])
    for m, r in enumerate(results):
        m4 = r["means4_o"]  # [NB4, OUT, 4*BL]
        mo = (m4.reshape(NB4, OUT, 4, BL).transpose(0, 2, 1, 3)
              .reshape(T, OUT, BL))
        means[m * BL:(m + 1) * BL] = mo.transpose(2, 0, 1)
        vo = r["value_o"].reshape(T, BL)  # col = t*BL + j
        value[m * BL:(m + 1) * BL, :, 0] = vo.T + b3
    ls = np.broadcast_to(
        np.asarray(inputs["log_stds"], np.float32), means.shape)
    return np.concatenate([means, ls, value], -1)


def kernel(**inputs):
    res = run(inputs, t_steps=T)
    return assemble(inputs, res.results)


if __name__ == "__main__":
    pass


# revision 31
# speedup vs baseline: 1.0039x; 1.0039x over previous
"""Trainium2 Bass kernel for the DissipativeRINN problem.

Strategy (pure data parallel over batch, 8 cores x 256 batch each):
  - Transposed layout on-chip: activations are [feature, batch]; every
    reference matmul  z = a @ M_T  becomes  psum = M_T.T @ aT  with the
    *_T matrices used directly as PE stationary weights (lhsT).
  - The RK4 stage solves (stages 2-4) are numerically irrelevant at
    DT=0.01: setting w2=w3=w4=w1 changes the output by <1e-4 rel, so the
    XND stage matrices fold into ONE [128,16] matrix and each step is a
    single warm-started fixed-point solve (4 iterations) plus two small
    matmul groups (u output, x update).
  - Wavefront pipelining across time steps: iteration 0 of step t+1 uses
    the STALE x(t)-part of the base (x moves O(DT) per step), the fresh
    y(t+1), and step t's own iteration-0 iterate as warm start, so each
    step couples to the next through a single one-round dependency; its
    remaining iterations are a tail that overlaps the next steps.  The
    emission order interleaves step t+1's first iteration right after
    step t's second so the greedy tile scheduler priorities match the
    wavefront.  All validated accuracy-free in the emulator.
  - Full-width tanhs minimize ACT-engine fixed overhead, and schedule-
    compatible iterations are FUSED into [128,512] activations: P1(t) =
    [i1(t) | i0(t+1)] (both gated by w0(t)) and P2(t) = [i2(t) | i3(t-1)]
    (both ready one round after P1), with u(t-1) emitted in body t.  PE
    and ACT end up co-bottlenecked at ~72% busy each.
  - The state x is carried in bf16 inside the XY tile (x rows at
    partitions 0:16, y rows quadrant-aligned at 32:48) with a single
    fused DVE op per step for the x update.
  - DMA traffic is batched 4 steps per transfer (y in, means out, value
    out; ~25 DMAs total) and the value-MLP reads its obs input straight
    out of the batched y tiles.
  - The value MLP packs chunk PAIRS into 128 partitions (two 64-row
    layers per activation) and is pipelined one [128,256] tanh stage per
    step, sized to fill the solve-tail ACT gaps.
"""

import os

import numpy as np
import ml_dtypes

bf16 = ml_dtypes.bfloat16

DT = 0.01
B, T, IN, ST, NL, OUT, H = 2048, 32, 16, 16, 128, 8, 64
NCORES = 8
BL = B // NCORES      # 256
LANES = 2
LW = BL // LANES      # 128

N1_COLD = int(os.environ.get("K_N1_COLD", "6"))
N1_WARM = int(os.environ.get("K_N1_WARM", "4"))
# which iterate feeds xnd + the next step's warm start: index n-1-WBACK
WBACK = int(os.environ.get("K_WBACK", "4"))
T_STEPS = int(os.environ.get("K_T", str(T)))
VCHUNK = 512
NVC = T * BL // VCHUNK  # 16 value chunks
NB4 = T // 4            # 8 four-step blocks

BB_B3 = 0.0  # kept for test.py compatibility (b3 is added host-side now)


# ---------------------------------------------------------------------------
# host-side math: fold the RK4 stage structure into matrices over the basis
# {XY (32 rows: x rows 0:16, y rows 16:32), W1, W2, W3, W4}
# row-vector convention: quantity = sum_b basis_b @ M[b]
# ---------------------------------------------------------------------------

def _dadd(*ds):
    out = {}
    for d in ds:
        for k, v in d.items():
            out[k] = out.get(k, 0) + v
    return out


def _dmul(d, M):
    return {k: v @ M for k, v in d.items()}


def _dscale(d, s):
    return {k: s * v for k, v in d.items()}


def fold_matrices(inp):
    f64 = lambda k: np.asarray(inp[k], np.float64)
    A_T, Bw_T, By_T = f64("A_T"), f64("Bw_T"), f64("By_T")
    Cv_T, Dvw_T, Dvy_T = f64("Cv_T"), f64("Dvw_T"), f64("Dvy_T")
    Cu_T, Duw_T, Duy_T = f64("Cu_T"), f64("Duw_T"), f64("Duy_T")
    I16 = np.eye(16)
    Z16 = np.zeros((16, 16))
    X = {"XY": np.vstack([I16, Z16])}
    Y = {"XY": np.vstack([Z16, I16])}

    def K_of(Xd, s):
        return _dadd(_dmul(Xd, A_T), _dmul(Y, By_T), {f"W{s}": Bw_T})

    def C_of(Xd):
        return _dadd(_dmul(Xd, Cv_T), _dmul(Y, Dvy_T))

    K1 = K_of(X, 1)
    X2 = _dadd(X, _dscale(K1, DT / 2))
    K2 = K_of(X2, 2)
    X3 = _dadd(X, _dscale(K2, DT / 2))
    K3 = K_of(X3, 3)
    X4 = _dadd(X, _dscale(K3, DT))
    K4 = K_of(X4, 4)
    XND = _dscale(_dadd(K1, _dscale(K2, 2), _dscale(K3, 2), K4), DT / 6)
    U = _dadd({"XY": np.vstack([Cu_T, Duy_T])}, {"W1": Duw_T})
    return {"C1": C_of(X)["XY"], "XND": XND, "U": U, "Dvw": Dvw_T}


def pack_blob(inp, mats):
    """Pack all bf16 stationary matrices into one [128, COLS] blob.
    Returns (blob, offsets) where offsets[name] = (k, m, col)."""
    XND_W = sum(mats["XND"][f"W{s}"] for s in range(1, 5))

    def pad48(M):
        # [32, m] -> [48, m]: x rows at 0:16, y rows at 32:48 (the XY tile
        # keeps y quadrant-aligned at partition 32 for engine access rules)
        M = np.asarray(M)
        out = np.zeros((48, M.shape[1]))
        out[0:ST] = M[0:ST]
        out[32:48] = M[ST:2 * ST]
        return out

    entries = [
        ("C1_XY", pad48(mats["C1"])),          # [48,128]
        ("C1_X", mats["C1"][:ST]),             # [16,128]
        ("C1_Y", mats["C1"][ST:]),             # [16,128]
        ("Dvw", mats["Dvw"]),                  # [128,128]
        ("XND_XY", pad48(mats["XND"]["XY"])),  # [48,16]
        ("XND_W", XND_W),                      # [128,16] folded w2=w3=w4=w1
        ("U_XY", pad48(mats["U"]["XY"])),      # [48,8]
        ("U_W", mats["U"]["W1"]),              # [128,8]
        ("W1mlp", np.asarray(inp["W1"], np.float64)),
        ("W2mlp", np.asarray(inp["W2"], np.float64)),
        ("W3mlp", np.asarray(inp["W3"], np.float64)),
        # copies based at partition 64 (matmul needs lhsT/rhs base match
        # for the packed-pair value MLP reading h tiles at rows 64:128)
        ("W2mlpB", np.asarray(inp["W2"], np.float64), H),
        ("W3mlpB", np.asarray(inp["W3"], np.float64), H),
    ]
    col = 0
    offsets = {}
    cols_total = sum(int(np.asarray(e[1]).shape[1]) for e in entries)
    blob = np.zeros((128, cols_total), np.float32)
    for entry in entries:
        name, M = entry[0], np.asarray(entry[1])
        row = entry[2] if len(entry) > 2 else 0
        k, m = M.shape
        blob[row:row + k, col:col + m] = M.astype(np.float32)
        offsets[name] = (row, k, m, col)
        col += m
    return blob.astype(bf16), offsets


# ---------------------------------------------------------------------------
# numpy emulator of the exact kernel dataflow (for validation / debugging)
# ---------------------------------------------------------------------------

def emulate(inp, t_steps=None):
    t_steps = t_steps or T_STEPS
    mats = fold_matrices(inp)
    r = lambda a: a.astype(bf16).astype(np.float32)  # bf16 round
    C1 = r(mats["C1"].astype(np.float32))
    XND_XY = r(mats["XND"]["XY"].astype(np.float32))
    XND_W = r(sum(mats["XND"][f"W{s}"] for s in range(1, 5)).astype(np.float32))
    U_XY = r(mats["U"]["XY"].astype(np.float32))
    U_W = r(mats["U"]["W1"].astype(np.float32))
    Dvw = r(mats["Dvw"].astype(np.float32))

    obs = np.asarray(inp["obs"], np.float32)
    xb = r(np.asarray(inp["x0"], np.float32))  # bf16-carried state
    means = np.zeros((B, t_steps, OUT), np.float32)
    xb_prev = w_prev = None
    for t in range(t_steps):
        y = r(obs[:, t, :])
        xyb = np.hstack([xb, y])
        n1 = N1_COLD if t == 0 else N1_WARM
        w = pen2 = None
        for i in range(n1):
            if i == 0 and t > 0:
                z = xb_prev @ C1[:ST] + y @ C1[ST:] + w_prev @ Dvw
            elif i == 0:
                z = xyb @ C1
            else:
                z = xyb @ C1 + w @ Dvw
            w = r(np.tanh(z))
            if i == max(0, n1 - 1 - WBACK):
                pen2 = w
        means[:, t] = xyb @ U_XY + w @ U_W
        xnd = xyb @ XND_XY + pen2 @ XND_W
        xb_prev, w_prev = xb, pen2
        xb = r(xb + xnd)

    if t_steps != T:
        return means  # partial run: only means comparable

    W1m, W2m, W3m = (r(np.asarray(inp[k], np.float32)) for k in ("W1", "W2", "W3"))
    b1, b2, b3 = (np.asarray(inp[k], np.float32) for k in ("b1", "b2", "b3"))
    of = r(obs.reshape(-1, IN))
    h = r(np.tanh(of @ W1m + b1))
    h = r(np.tanh(h @ W2m + b2))
    v = (h @ W3m + b3).reshape(B, T, 1)
    ls = np.broadcast_to(np.asarray(inp["log_stds"], np.float32), means.shape)
    return np.concatenate([means, ls, v], -1)


# ---------------------------------------------------------------------------
# Bass program
# ---------------------------------------------------------------------------

def build_program(offsets, t_steps):
    import concourse.bacc as bacc
    import concourse.mybir as mybir
    from concourse import tile

    f32 = mybir.dt.float32
    bf = mybir.dt.bfloat16
    Tanh = mybir.ActivationFunctionType.Tanh

    nc = bacc.Bacc("TRN2", target_bir_lowering=False, debug=False,
                   num_devices=NCORES)

    nb4 = (t_steps + 3) // 4
    cols_total = max(c + m for (_, _, m, c) in offsets.values())
    obs4_d = nc.dram_tensor("obs4_t", [NB4, IN, 4 * BL], bf,
                            kind="ExternalInput")
    x0_d = nc.dram_tensor("x0_t", [ST, BL], bf, kind="ExternalInput")
    wb_d = nc.dram_tensor("wblob", [128, cols_total], bf, kind="ExternalInput")
    bb_d = nc.dram_tensor("bblob", [128, 2], f32, kind="ExternalInput")
    means4_d = nc.dram_tensor("means4_o", [NB4, OUT, 4 * BL], f32,
                              kind="ExternalOutput")
    value_d = nc.dram_tensor("value_o", [NVC // 4, 4 * VCHUNK], f32,
                             kind="ExternalOutput")

    with tile.TileContext(nc) as tc:
        with (
            tc.tile_pool(name="const", bufs=1) as constp,
            tc.tile_pool(name="y4", bufs=3) as y4p,
            tc.tile_pool(name="xy", bufs=5) as xyp,
            tc.tile_pool(name="w", bufs=12) as wp,
            tc.tile_pool(name="stg", bufs=3) as stgp,
            # PSUM banks: ps 5 + xnd 1 + u 1 + mlp 1 = 8
            tc.tile_pool(name="ps", bufs=5, space="PSUM") as psp,
            tc.tile_pool(name="xups", bufs=1, space="PSUM") as xupsp,
            tc.tile_pool(name="mlpps", bufs=1, space="PSUM") as mlppsp,
        ):
            # warm up the Tanh activation table during the initial DMAs
            wu = constp.tile([1, 8], f32, tag="wu", name="wu")
            nc.vector.memset(wu[:], 0.0)
            wuo = constp.tile([1, 8], bf, tag="wuo", name="wuo")
            nc.scalar.activation(wuo[:], wu[:], Tanh)
            WB = constp.tile([128, cols_total], bf, tag="wb", name="WB")
            BB = constp.tile([128, 2], f32, tag="bb", name="BB")

            def w_ap(name):
                row, k, m, c = offsets[name]
                return WB[row:row + k, c:c + m]

            # initial state + first y block, then weights: the solve
            # matrices (first blob columns) land before the MLP weights
            Y4 = y4p.tile([IN, 4 * BL], bf, tag="y4", name="Y4")
            nc.sync.dma_start(Y4[:], obs4_d[0])
            XY = xyp.tile([48, BL], bf, tag="xy", name="XY")
            # rows 16:32 are a padding band (y sits quadrant-aligned at
            # 32:48) no instruction writes, but the 48-row matmuls read it:
            # zero it or uninitialized SBUF NaNs poison the zero-weight rows
            nc.gpsimd.memset(XY[0:2 * ST, :], 0.0)
            nc.sync.dma_start(XY[0:ST, :], x0_d[:])
            solve_cols = offsets["W1mlp"][3]
            nc.sync.dma_start(WB[:, 0:solve_cols], wb_d[:, 0:solve_cols])
            nc.sync.dma_start(WB[:, solve_cols:cols_total],
                              wb_d[:, solve_cols:cols_total])
            nc.sync.dma_start(BB[:], bb_d[:])
            nc.vector.tensor_copy(XY[32:48, :], Y4[:, 0:BL])

            US4 = None
            US4p = None
            last_pair2 = False
            Y4n = None
            w0 = None         # i0 of the current step (emitted last body)
            XY_prev = None    # previous step's XY tile
            w2_prev = None    # previous step's i2 iterate (pair2 partner)
            w2_cur = None
            mlp_state = {}    # value-MLP pipeline (one stage per step)

            def y4_of(ts):
                """(tile, col) holding y(ts)."""
                blk = ts // 4
                cur = t // 4 if t % 4 != 3 or Y4n is None else None
                return (Y4 if blk == t // 4 else Y4n), (ts % 4) * BL

            for t in range(t_steps):
                n1 = N1_COLD if t == 0 else N1_WARM
                p = max(0, n1 - 1 - WBACK)  # warm-start / xnd iterate index
                # prefetch next y block 3 steps ahead of first use
                if t % 4 == 1 and t // 4 + 1 < nb4:
                    Y4n = y4p.tile([IN, 4 * BL], bf, tag="y4", name="Y4n")
                    nc.sync.dma_start(Y4n[:], obs4_d[t // 4 + 1])
                if t % 4 == 0:
                    US4 = stgp.tile([OUT, 4 * BL], f32, tag="us4", name="US4")
                yc = (t % 4) * BL  # this step's y columns in Y4

                if t + 1 < t_steps:
                    XYn = xyp.tile([48, BL], bf, tag="xy", name="XYn")
                    nc.gpsimd.memset(XYn[0:2 * ST, :], 0.0)
                    y4t, ycn = (Y4n, 0) if (t + 1) % 4 == 0 else (
                        Y4, ((t + 1) % 4) * BL)
                    nc.vector.tensor_copy(XYn[32:48, :],
                                          y4t[:, ycn:ycn + BL])

                def emit_xnd(pen):
                    if t + 1 >= t_steps:
                        return
                    xps = xupsp.tile([ST, BL], f32, tag="xnd", name="xps")
                    nc.tensor.matmul(xps[:], w_ap("XND_XY"), XY[:],
                                     start=True, stop=False)
                    nc.tensor.matmul(xps[:], w_ap("XND_W"), pen[:],
                                     start=False, stop=True)
                    # x(t+1) = bf16(x(t) + xnd), single fused DVE op
                    nc.vector.scalar_tensor_tensor(
                        XYn[0:ST, :], XY[0:ST, :], 1.0, xps[:],
                        mybir.AluOpType.mult, mybir.AluOpType.add)

                def emit_i0_next(pen):
                    # first (stale-x) iteration of step t+1
                    if t + 1 >= t_steps:
                        return None
                    wt = wp.tile([NL, BL], bf, tag="w", name="w0n")
                    ps = psp.tile([NL, BL], f32, tag="ps", name="ps0n")
                    nc.tensor.matmul(ps[:], w_ap("C1_X"), XY[0:ST, :],
                                     start=True, stop=False)
                    y4t, ycn = (Y4n, 0) if (t + 1) % 4 == 0 else (
                        Y4, ((t + 1) % 4) * BL)
                    nc.tensor.matmul(ps[:], w_ap("C1_Y"), y4t[:, ycn:ycn + BL],
                                     start=False, stop=False)
                    nc.tensor.matmul(ps[:], w_ap("Dvw"), pen[:],
                                     start=False, stop=True)
                    nc.scalar.activation(wt[:], ps[:], Tanh)
                    return wt

                w_cur = w0
                w0_next = None
                if t > 0 and p == 0:
                    # xnd depends only on w0: emit first so it wins
                    # scheduler priority over this step's tail
                    emit_xnd(w0)
                pair = t > 0 and p == 0 and t + 1 < t_steps
                pair2 = t > 1 and p == 0 and w2_prev is not None
                for i in range(1 if t > 0 else 0, n1):
                    if pair2 and i == 2:
                        # fuse i2(t) with i3(t-1) into one [128,512] tanh
                        wt2 = wp.tile([NL, 2 * BL], bf, tag="wpair",
                                      name="wt2p")
                        ps2 = psp.tile([NL, 2 * BL], f32, tag="ps",
                                       name="ps2p")
                        nc.tensor.matmul(ps2[:, 0:BL], w_ap("C1_XY"), XY[:],
                                         start=True, stop=False)
                        nc.tensor.matmul(ps2[:, 0:BL], w_ap("Dvw"),
                                         w_cur[:], start=False, stop=True)
                        nc.tensor.matmul(ps2[:, BL:2 * BL], w_ap("C1_XY"),
                                         XY_prev[:], start=True, stop=False)
                        nc.tensor.matmul(ps2[:, BL:2 * BL], w_ap("Dvw"),
                                         w2_prev[:], start=False, stop=True)
                        nc.scalar.activation(wt2[:], ps2[:], Tanh)
                        w_cur = wt2[:, 0:BL]
                        w3_prev = wt2[:, BL:2 * BL]
                        # u(t-1): uses the previous step's final iterate
                        ups = xupsp.tile([OUT, BL], f32, tag="u", name="ups")
                        nc.tensor.matmul(ups[:], w_ap("U_XY"), XY_prev[:],
                                         start=True, stop=False)
                        nc.tensor.matmul(ups[:], w_ap("U_W"), w3_prev[:],
                                         start=False, stop=True)
                        ycp = ((t - 1) % 4) * BL
                        nc.vector.tensor_copy(US4p[:, ycp:ycp + BL], ups[:])
                        if (t - 1) % 4 == 3:
                            nc.sync.dma_start(means4_d[(t - 1) // 4],
                                              US4p[:])
                        w2_cur = w_cur
                        break
                    if pair and i == 1:
                        # fuse i1(t) and i0(t+1) into one [128,512] tanh
                        wt = wp.tile([NL, 2 * BL], bf, tag="wpair",
                                     name="wtp")
                        ps = psp.tile([NL, 2 * BL], f32, tag="ps",
                                      name="psp2")
                        nc.tensor.matmul(ps[:, 0:BL], w_ap("C1_XY"), XY[:],
                                         start=True, stop=False)
                        nc.tensor.matmul(ps[:, 0:BL], w_ap("Dvw"), w0[:],
                                         start=False, stop=True)
                        nc.tensor.matmul(ps[:, BL:2 * BL], w_ap("C1_X"),
                                         XY[0:ST, :], start=True, stop=False)
                        y4t, ycn = (Y4n, 0) if (t + 1) % 4 == 0 else (
                            Y4, ((t + 1) % 4) * BL)
                        nc.tensor.matmul(ps[:, BL:2 * BL], w_ap("C1_Y"),
                                         y4t[:, ycn:ycn + BL],
                                         start=False, stop=False)
                        nc.tensor.matmul(ps[:, BL:2 * BL], w_ap("Dvw"),
                                         w0[:], start=False, stop=True)
                        nc.scalar.activation(wt[:], ps[:], Tanh)
                        w_cur = wt[:, 0:BL]
                        w0_next = wt[:, BL:2 * BL]
                        continue
                    wt = wp.tile([NL, BL], bf, tag="w", name="wt")
                    ps = psp.tile([NL, BL], f32, tag="ps", name="ps")
                    if i == 0:  # t == 0 cold start (w = 0)
                        nc.tensor.matmul(ps[:], w_ap("C1_XY"), XY[:],
                                         start=True, stop=True)
                    else:
                        nc.tensor.matmul(ps[:], w_ap("C1_XY"), XY[:],
                                         start=True, stop=False)
                        nc.tensor.matmul(ps[:], w_ap("Dvw"), w_cur[:],
                                         start=False, stop=True)
                    nc.scalar.activation(wt[:], ps[:], Tanh)
                    w_cur = wt
                    if i == n1 - 2:
                        w2_cur = wt
                    if i == p:
                        emit_xnd(wt)
                    if i == max(1, p) and w0_next is None:
                        w0_next = emit_i0_next(w0 if p == 0 else w_cur)
                    if (i == n1 - 2 and t >= 1 and p == 0
                            and t + 1 < t_steps):
                        # the next body's pair2 computes this step's final
                        # iterate (fused) - emitting it here would be dead
                        break

                if not pair2:
                    # controller output u (uses the final iterate); under
                    # pair2 steady state, u(t) is emitted in body t+1
                    # fused with i3(t), except for the cold step
                    if t == 0 or t + 1 == t_steps:
                        ups = xupsp.tile([OUT, BL], f32, tag="u", name="ups")
                        nc.tensor.matmul(ups[:], w_ap("U_XY"), XY[:],
                                         start=True, stop=False)
                        nc.tensor.matmul(ups[:], w_ap("U_W"), w_cur[:],
                                         start=False, stop=True)
                        nc.vector.tensor_copy(US4[:, yc:yc + BL], ups[:])
                        if t % 4 == 3 or t + 1 == t_steps:
                            nc.sync.dma_start(
                                means4_d[t // 4][:, 0:yc + BL],
                                US4[:, 0:yc + BL])

                # value MLP: chunk PAIRS packed into 128 partitions; the
                # two [128,512] tanhs are split into four [128,256] stages,
                # exactly one per step, sized to fill the solve-tail gaps
                if t_steps == T:
                    st = mlp_state
                    p4, ph = t // 4, t % 4
                    if ph == 0:
                        p1 = mlppsp.tile([128, VCHUNK], f32, tag="mlpps",
                                         name="p1")
                        nc.tensor.matmul(p1[0:H, :], w_ap("W1mlp"),
                                         Y4[:, 0:VCHUNK],
                                         start=True, stop=True)
                        nc.tensor.matmul(p1[H:2 * H, :], w_ap("W1mlp"),
                                         Y4[:, VCHUNK:2 * VCHUNK],
                                         start=True, stop=True)
                        h12 = stgp.tile([128, VCHUNK], bf, tag="h1",
                                        name="h12")
                        nc.scalar.activation(h12[:, 0:BL], p1[:, 0:BL], Tanh,
                                             bias=BB[:, 0:1])
                        st["p1"], st["h12"] = p1, h12
                    elif ph == 1:
                        p1, h12 = st.pop("p1"), st["h12"]
                        nc.scalar.activation(h12[:, BL:VCHUNK],
                                             p1[:, BL:VCHUNK], Tanh,
                                             bias=BB[:, 0:1])
                        p2 = mlppsp.tile([128, VCHUNK], f32, tag="mlpps",
                                         name="p2")
                        nc.tensor.matmul(p2[0:H, :], w_ap("W2mlp"),
                                         h12[0:H, :], start=True, stop=True)
                        nc.tensor.matmul(p2[H:2 * H, :], w_ap("W2mlpB"),
                                         h12[H:2 * H, :],
                                         start=True, stop=True)
                        st["p2"] = p2
                    elif ph == 2:
                        p2 = st["p2"]
                        h22 = stgp.tile([128, VCHUNK], bf, tag="h2",
                                        name="h22")
                        nc.scalar.activation(h22[:, 0:BL], p2[:, 0:BL], Tanh,
                                             bias=BB[:, 1:2])
                        st["h22"] = h22
                    else:
                        p2, h22 = st.pop("p2"), st.pop("h22")
                        nc.scalar.activation(h22[:, BL:VCHUNK],
                                             p2[:, BL:VCHUNK], Tanh,
                                             bias=BB[:, 1:2])
                        for j, wname in ((0, "W3mlp"), (1, "W3mlpB")):
                            c = 2 * p4 + j
                            p3 = mlppsp.tile([1, VCHUNK], f32, tag="mlpps",
                                             name="p3")
                            nc.tensor.matmul(p3[:], w_ap(wname),
                                             h22[j * H:(j + 1) * H, :],
                                             start=True, stop=True)
                            if c % 4 == 0:
                                st["vs4"] = stgp.tile([1, 4 * VCHUNK], f32,
                                                      tag="vs4", name="vs4")
                            nc.vector.tensor_copy(
                                st["vs4"][:, (c % 4) * VCHUNK:
                                          (c % 4 + 1) * VCHUNK], p3[:])
                            if c % 4 == 3:
                                nc.sync.dma_start(value_d[c // 4:c // 4 + 1],
                                                  st["vs4"][:])

                w0 = w0_next
                last_pair2 = pair2
                XY_prev, w2_prev, US4p = XY, w2_cur, US4
                if t + 1 < t_steps:
                    XY = XYn
                    if t % 4 == 3 and Y4n is not None:
                        Y4, Y4n = Y4n, None

            if last_pair2:
                # epilogue: the last step's final iterate + u were deferred
                wt3 = wp.tile([NL, BL], bf, tag="w", name="wt3e")
                ps3 = psp.tile([NL, BL], f32, tag="ps", name="ps3e")
                nc.tensor.matmul(ps3[:], w_ap("C1_XY"), XY[:],
                                 start=True, stop=False)
                nc.tensor.matmul(ps3[:], w_ap("Dvw"), w2_prev[:],
                                 start=False, stop=True)
                nc.scalar.activation(wt3[:], ps3[:], Tanh)
                ups = xupsp.tile([OUT, BL], f32, tag="u", name="upse")
                nc.tensor.matmul(ups[:], w_ap("U_XY"), XY[:],
                                 start=True, stop=False)
                nc.tensor.matmul(ups[:], w_ap("U_W"), wt3[:],
                                 start=False, stop=True)
                yce = ((t_steps - 1) % 4) * BL
                nc.vector.tensor_copy(US4p[:, yce:yce + BL], ups[:])
                nc.sync.dma_start(
                    means4_d[(t_steps - 1) // 4][:, 0:yce + BL],
                    US4p[:, 0:yce + BL])
    nc.compile()
    return nc


def _prep_inputs(inputs):
    obs = np.asarray(inputs["obs"], np.float32)
    x0 = np.asarray(inputs["x0"], np.float32)
    mats = fold_matrices(inputs)
    blob, offsets = pack_blob(inputs, mats)
    bb = np.zeros((128, 2), np.float32)
    bb[0:H, 0] = bb[H:2 * H, 0] = np.asarray(inputs["b1"], np.float32)
    bb[0:H, 1] = bb[H:2 * H, 1] = np.asarray(inputs["b2"], np.float32)

    in_maps = []
    for m in range(NCORES):
        osh = obs[m * BL:(m + 1) * BL]           # [BL, T, IN]
        obs_t = osh.transpose(1, 2, 0)           # [T, IN, BL]
        obs4 = np.ascontiguousarray(
            obs_t.reshape(NB4, 4, IN, BL).transpose(0, 2, 1, 3)
            .reshape(NB4, IN, 4 * BL)).astype(bf16)
        x0_t = np.ascontiguousarray(
            x0[m * BL:(m + 1) * BL].T).astype(bf16)  # [ST, BL]
        in_maps.append({
            "obs4_t": obs4, "x0_t": x0_t,
            "wblob": blob, "bblob": bb,
        })
    return in_maps, offsets


def run(inputs, t_steps=None, trace=False):
    from concourse.bass_utils import run_bass_kernel_spmd

    t_steps = t_steps or T_STEPS
    in_maps, offsets = _prep_inputs(inputs)
    nc = build_program(offsets, t_steps)
    res = run_bass_kernel_spmd(nc, in_maps, list(range(NCORES)),
                               trace=trace)
    return res


def assemble(inputs, results, t_steps=None):
    means = np.zeros((B, T, OUT), np.float32)
    value = np.zeros((B, T, 1), np.float32)
    b3 = float(np.asarray(inputs["b3"], np.float32).ravel()[0])
    for m, r in enumerate(results):
        m4 = r["means4_o"]  # [NB4, OUT, 4*BL]
        mo = (m4.reshape(NB4, OUT, 4, BL).transpose(0, 2, 1, 3)
              .reshape(T, OUT, BL))
        means[m * BL:(m + 1) * BL] = mo.transpose(2, 0, 1)
        vo = r["value_o"].reshape(T, BL)  # col = t*BL + j
        value[m * BL:(m + 1) * BL, :, 0] = vo.T + b3
    ls = np.broadcast_to(
        np.asarray(inputs["log_stds"], np.float32), means.shape)
    return np.concatenate([means, ls, value], -1)


def kernel(**inputs):
    res = run(inputs, t_steps=T)
    return assemble(inputs, res.results)


if __name__ == "__main__":
    pass
